# revision 8
# baseline (speedup 1.0000x reference)
"""FP8 per-tensor dynamic-quantized Linear on 8 TRN2 NeuronCores.

Computes reference semantics of:
    x2 = x.reshape(-1, 4096)
    x_fp8, s_i = quantize_e4m3fn(x2)      # per-tensor amax -> scale
    w_fp8, s_w = quantize_e4m3fn(weight)
    out = (x_fp8.f32 @ w_fp8.f32.T) * (s_i * s_w) + bias

Sharding: token-parallel. Each core owns 1024 tokens of x (stored k-major,
DoubleRow pair-interleaved), the full weight (blocked layout), and a distinct
1/8 slice of the weight rows for the distributed amax pass. Global per-tensor
amax for x and w = local absmax reduce + one 8-core AllReduce(max) of 2
floats.

TRN fp8_e4m3 saturates at +-240 (OCP e4m3fn goes to 448), so the device
quantizes with multiplier 224/amax == (448/amax)/2 exactly (power-of-two =>
bit-identical RNE mantissa rounding vs the reference) and the matmul output
is rescaled by (amax_x*amax_w)/50176 = 4*s_i*s_w to compensate the halvings.

Matmul runs in fp8 DoubleRow mode: stationary W tile [128,2,128] plane-major,
moving x tile pair-interleaved ([p, t, 2] memory viewed as [p, 2, t]) which
the PE streams at 2 fp8/cycle, accumulating 16 k-blocks into PSUM. Epilogue =
single ScalarE activation (scale + per-partition bias). Output is written
transposed [n, t] per core and fixed up on the host.
"""

import numpy as np

import concourse.bass as bass
import concourse.mybir as mybir
import concourse.tile as tile
from concourse import bacc, bass_isa, bass_utils

N_CORES = 8
XB, XS, K = 4, 2048, 4096   # x: [4, 2048, 4096]
N = 4096                    # weight: [N, K]
T = XB * XS                 # 8192 tokens
TC = T // N_CORES           # 1024 tokens per core
NB = N // 128               # 32 output-feature blocks
KB = K // 128               # 32 k subtiles of 128
KBB = K // 256              # 16 DoubleRow k blocks
TFREE = 512
TT = TC // TFREE            # 2 token tiles per core
WS = N // N_CORES           # 512 weight rows per core for amax

F32 = mybir.dt.float32
FP8 = mybir.dt.float8e4
AX = mybir.AxisListType.X
OP = mybir.AluOpType
ACTF = mybir.ActivationFunctionType

_cache: dict = {}


def _emit(tc, nc, xt_d, wt_d, ws_d, b_d, out_d):
    with tc.tile_pool(name="stat", bufs=1) as stat, \
         tc.tile_pool(name="x8p", bufs=1) as x8p, \
         tc.tile_pool(name="dram", bufs=1, space="DRAM") as dram:

        pm = stat.tile([128, KBB + 8], F32)
        bias_sb = stat.tile([128, NB], F32)
        nc.sync.dma_start(bias_sb[:], b_d[:])

        # resident quantized x, pair-interleaved:
        # x8[p, kbb, t*2+i] = q_x * x[token t, k=256*kbb+128*i+p]
        x8 = x8p.tile([128, KBB, 2 * TC], FP8)

        qsc = stat.tile([128, 2], F32)
        scomb = stat.tile([128, 1], F32)

        # ---- phase A: stream x shard (kept resident) + w slice, absmax reduce
        with tc.tile_pool(name="xap", bufs=KBB) as xap, \
             tc.tile_pool(name="wsp", bufs=2) as wsp:
            xa_tiles = []
            for kbb in range(KBB):
                xa = xap.tile([128, 2 * TC], F32, name=f"xa{kbb}", tag="xa")
                nc.sync.dma_start(xa[:], xt_d[kbb])
                nc.vector.tensor_reduce(pm[:, kbb:kbb + 1], xa[:], AX, OP.max,
                                        apply_absolute_value=True)
                xa_tiles.append(xa)
            for i in range(4):
                for h in range(2):
                    wsl = wsp.tile([128, 2048], F32, name="wsl", tag="wsl")
                    nc.gpsimd.dma_start(
                        wsl[:],
                        ws_d[i * 128:(i + 1) * 128, h * 2048:(h + 1) * 2048])
                    c = KBB + i * 2 + h
                    nc.vector.tensor_reduce(pm[:, c:c + 1], wsl[:], AX, OP.max,
                                            apply_absolute_value=True)

            am = stat.tile([128, 2], F32)
            nc.vector.tensor_reduce(am[:, 0:1], pm[:, 0:KBB], AX, OP.max)
            nc.vector.tensor_reduce(am[:, 1:2], pm[:, KBB:KBB + 8], AX, OP.max)
            amr = stat.tile([128, 2], F32)
            nc.gpsimd.partition_all_reduce(amr[:], am[:], channels=128,
                                           reduce_op=bass_isa.ReduceOp.max)

            # ---- cross-core AllGather of (amax_x, amax_w), then local max.
            # AllGather has ~2x lower latency than AllReduce on this fabric.
            cin = dram.tile([1, 2], F32)
            cout = dram.tile([N_CORES, 2], F32)
            nc.sync.dma_start(cin[:], amr[0:1, :])
            nc.gpsimd.collective_compute(
                "AllGather", OP.bypass,
                replica_groups=[list(range(N_CORES))],
                ins=[cin.opt()], outs=[cout.opt()],
            )
            g1 = stat.tile([N_CORES, 2], F32)
            nc.sync.dma_start(g1[:], cout[:])
            g1r = stat.tile([N_CORES, 2], F32)
            nc.gpsimd.partition_all_reduce(g1r[:], g1[:], channels=N_CORES,
                                           reduce_op=bass_isa.ReduceOp.max)
            gam = stat.tile([128, 2], F32)
            nc.gpsimd.partition_broadcast(gam[:], g1r[:], channels=128)
            nc.vector.tensor_scalar_max(gam[:], gam[:], 1e-12)

            # scales: rec ~= 1/amax (reciprocal + 1 Newton step), q = 224*rec,
            # scomb = amax_x*amax_w/50176  (= s_i*s_w*4)
            rec = stat.tile([128, 2], F32)
            tmp = stat.tile([128, 2], F32)
            nc.vector.reciprocal(rec[:], gam[:])
            nc.vector.tensor_tensor(tmp[:], gam[:], rec[:], OP.mult)
            nc.vector.tensor_scalar(tmp[:], tmp[:], -1.0, 2.0, OP.mult, OP.add)
            nc.vector.tensor_tensor(rec[:], rec[:], tmp[:], OP.mult)
            nc.vector.tensor_scalar_mul(qsc[:], rec[:], 224.0)
            nc.vector.tensor_tensor(scomb[:], gam[:, 0:1], gam[:, 1:2], OP.mult)
            nc.vector.tensor_scalar_mul(scomb[:], scomb[:], 1.0 / 50176.0)

            # ---- phase B: quantize resident x (frees xa slots in kbb order)
            for kbb in range(KBB):
                if kbb % 2 == 0:
                    nc.vector.tensor_scalar_mul(x8[:, kbb, :], xa_tiles[kbb][:],
                                                qsc[:, 0:1])
                else:
                    nc.scalar.activation(x8[:, kbb, :], xa_tiles[kbb][:],
                                         ACTF.Copy, scale=qsc[:, 0:1])

            # ---- phase C: stream W blocks (reusing freed xa slots), quantize,
            # DoubleRow matmul, epilogue
            with tc.tile_pool(name="w8p", bufs=3) as w8p, \
                 tc.tile_pool(name="psp", bufs=4, space="PSUM") as psp, \
                 tc.tile_pool(name="obp", bufs=4) as obp:
                _matmul_phase(tc, nc, xap, wsp, w8p, psp, obp, wt_d, out_d,
                              x8, qsc, scomb, bias_sb)


def _matmul_phase(tc, nc, xap, wsp, w8p, psp, obp, wt_d, out_d, x8, qsc,
                  scomb, bias_sb):
    for j in range(NB):
        w8 = w8p.tile([128, KB, 128], FP8, name="w8", tag="w8")
        for h in range(2):
            # j=0 stages through the freed ws slots (same 8KB/partition size)
            # so its W stream overlaps the collective window; later blocks
            # reuse xa slots as the x-quantize frees them.
            if j == 0:
                wfh = wsp.tile([128, KB // 2, 128], F32, name=f"wf{j}_{h}",
                               tag="wsl")
            else:
                wfh = xap.tile([128, KB // 2, 128], F32, name=f"wf{j}_{h}",
                               tag="xa")
            nc.sync.dma_start(wfh[:],
                              wt_d[j, :, h * (KB // 2):(h + 1) * (KB // 2), :])
            nc.vector.tensor_scalar_mul(
                w8[:, h * (KB // 2):(h + 1) * (KB // 2), :], wfh[:],
                qsc[:, 1:2])
        pts = [psp.tile([128, TFREE], F32, name=f"pt{tt}", tag=f"pt{tt}")
               for tt in range(TT)]
        for kbb in range(KBB):
            lhs = w8[:, 2 * kbb:2 * kbb + 2, :]
            for tt in range(TT):
                rhs = x8[:, kbb, tt * 2 * TFREE:(tt + 1) * 2 * TFREE]
                rhs = rhs.rearrange("p (t two) -> p two t", two=2)
                nc.tensor.matmul(pts[tt][:], lhs, rhs,
                                 start=(kbb == 0), stop=(kbb == KBB - 1),
                                 perf_mode=mybir.MatmulPerfMode.DoubleRow)
        for tt in range(TT):
            ob = obp.tile([128, TFREE], F32, name="ob", tag="ob")
            nc.scalar.activation(ob[:], pts[tt][:], ACTF.Identity,
                                 bias=bias_sb[:, j:j + 1], scale=scomb[:])
            nc.gpsimd.dma_start(
                out_d[j * 128:(j + 1) * 128, tt * TFREE:(tt + 1) * TFREE],
                ob[:])


def _build():
    nc = bacc.Bacc("TRN2", target_bir_lowering=False, debug=False,
                   enable_asserts=False, num_devices=N_CORES)
    xt_d = nc.dram_tensor("xt", [KBB, 128, 2 * TC], F32, kind="ExternalInput").ap()
    wt_d = nc.dram_tensor("wt", [NB, 128, KB, 128], F32, kind="ExternalInput").ap()
    ws_d = nc.dram_tensor("ws", [WS, K], F32, kind="ExternalInput").ap()
    b_d = nc.dram_tensor("bias", [128, NB], F32, kind="ExternalInput").ap()
    out_d = nc.dram_tensor("out", [N, TC], F32, kind="ExternalOutput").ap()
    with tile.TileContext(nc) as tc:
        _emit(tc, nc, xt_d, wt_d, ws_d, b_d, out_d)
    nc.compile()
    return nc


def _prepare_inputs(x, weight, bias):
    x = np.ascontiguousarray(np.asarray(x, dtype=np.float32))
    weight = np.ascontiguousarray(np.asarray(weight, dtype=np.float32))
    bias = np.ascontiguousarray(np.asarray(bias, dtype=np.float32))

    x2 = x.reshape(T, K)
    # weight [N, K] -> blocked W^T: [j, p, kb, n] = weight[j*128+n, kb*128+p]
    wt = np.ascontiguousarray(
        weight.reshape(NB, 128, KB, 128).transpose(0, 3, 2, 1))
    bias_dev = np.ascontiguousarray(bias.reshape(NB, 128).T)  # [128, NB]

    in_maps = []
    for c in range(N_CORES):
        xs = x2[c * TC:(c + 1) * TC, :]                  # [TC, K]
        # -> [kbb, p, t, i] with k = kbb*256 + i*128 + p, flattened (t,i)
        xdev = np.ascontiguousarray(
            xs.reshape(TC, KBB, 2, 128).transpose(1, 3, 0, 2)
        ).reshape(KBB, 128, 2 * TC)
        in_maps.append({
            "xt": xdev,
            "wt": wt,
            "ws": np.ascontiguousarray(weight[c * WS:(c + 1) * WS, :]),
            "bias": bias_dev,
        })
    return in_maps


def _run(x, weight, bias, trace=False):
    if "nc" not in _cache:
        _cache["nc"] = _build()
    nc = _cache["nc"]
    in_maps = _prepare_inputs(x, weight, bias)
    res = bass_utils.run_bass_kernel_spmd(
        nc, in_maps, core_ids=list(range(N_CORES)), trace=trace)
    out = np.empty((T, N), dtype=np.float32)
    for c in range(N_CORES):
        out[c * TC:(c + 1) * TC, :] = res.results[c]["out"].T
    return out.reshape(XB, XS, N), res


def kernel(x, weight, bias):
    out, _ = _run(x, weight, bias, trace=False)
    return out
